# revision 39
# baseline (speedup 1.0000x reference)
"""ALiBi attention (B=4, S=1024, D=1024, H=16) on 8 TRN2 NeuronCores.

Sharding: 8 cores = 4 batches x 2 head-groups (8 heads / 512 hidden each).
Each core computes, for its (batch, head-group):
    QT = wq.T @ xqT          [512, S]   (head-dim-major, "transposed" layout)
    KT = wq.T @ xkT          [512, S]
    V  = xvT.T @ wq          [S, 512]
    per head h:  ST[j,i] = KT_h.T @ QT_h          (scores transposed)
                 P = exp(ST) * T_h[., i-j]         (post-exp Toeplitz ALiBi)
                 ctxT_h = V_h.T @ P ;  sums = 1^T @ P  (PSUM-accumulated)
                 ctxT_h *= 1/sums  (broadcast along partitions)
    outT = wo.T @ ctxT       [1024, S]  (partial output, transposed, fp16)
Host transposes each core's outT and sums the two head-group partials.

ALiBi is applied AFTER exp as a multiply by a precomputed per-head
Toeplitz table T[jl, m] = exp(-slope * max(m - jl, 0)) (bf16, DVE 2x
mode, both heads of a pair in one instruction) instead of the fp32
scalar_tensor_tensor bias-add before exp -- this halves the DVE cost
and takes the bias off the scores->exp critical path.  The two heads'
score tiles land in adjacent PSUM banks so one ACTIVATE exps 1024
columns, amortizing the ACT per-instruction overhead (352 cycles).

Matmul operands are fp16 (bf16 for P/V, which need fp32-like range), so
every matmul streams at 1 cycle/row.  Mask input is all-ones per the
problem spec (where(mask==0) is the identity), so it is not shipped.
"""

import math
from contextlib import ExitStack

import numpy as np

B, S, D = 4, 1024, 1024
H, HD = 16, 64
HL = 8          # heads per core
DL = 512        # local hidden (= HL * HD)
NCORES = 8

_CACHE = {}


def _alibi_slopes(n_head):
    main = 2 ** int(math.log2(n_head))
    m_main = 2.0 ** (-8.0 / main)
    m = m_main ** np.arange(1, 1 + main, dtype=np.float32)
    if main < n_head:
        intra = 2.0 ** (-4.0 / main)
        extra = intra ** np.arange(1, 1 + 2 * (n_head - main), 2, dtype=np.float32)
        m = np.concatenate([m, extra])
    return m.astype(np.float32)


def _build_nc():
    import concourse.bass as bass
    import concourse.mybir as mybir
    import concourse.tile as tile
    from concourse import bacc

    f32 = mybir.dt.float32
    f16 = mybir.dt.float16
    bf16 = mybir.dt.bfloat16
    i32 = mybir.dt.int32
    EXP = mybir.ActivationFunctionType.Exp
    MULT = mybir.AluOpType.mult
    MAX = mybir.AluOpType.max

    nc = bacc.Bacc("TRN2", target_bir_lowering=False, debug=False,
                   num_devices=NCORES)

    xq = nc.dram_tensor("xq", [D, S], f16, kind="ExternalInput").ap()
    xk = nc.dram_tensor("xk", [D, S], f16, kind="ExternalInput").ap()
    xv = nc.dram_tensor("xv", [D, S], f16, kind="ExternalInput").ap()
    wq = nc.dram_tensor("wq", [D, DL], f16, kind="ExternalInput").ap()
    wo = nc.dram_tensor("wo", [DL, D], f16, kind="ExternalInput").ap()
    # negated per-head ALiBi slopes (this core's 8 heads)
    sl = nc.dram_tensor("sl", [1, HL], f32, kind="ExternalInput").ap()
    out = nc.dram_tensor("out", [D, S], f16, kind="ExternalOutput").ap()

    with ExitStack() as ctx:
        tc = ctx.enter_context(tile.TileContext(nc))

        consts = ctx.enter_context(tc.tile_pool(name="consts", bufs=1))
        xvp = ctx.enter_context(tc.tile_pool(name="xvp", bufs=1))
        xsp = ctx.enter_context(tc.tile_pool(name="xsp", bufs=1))
        big = ctx.enter_context(tc.tile_pool(name="big", bufs=1))
        pexp = ctx.enter_context(tc.tile_pool(name="pexp", bufs=3))
        small = ctx.enter_context(tc.tile_pool(name="small", bufs=2))
        mm_ps = ctx.enter_context(tc.tile_pool(name="mm_ps", bufs=2, space="PSUM"))
        sc_ps = ctx.enter_context(tc.tile_pool(name="sc_ps", bufs=2, space="PSUM"))
        pvs_ps = ctx.enter_context(tc.tile_pool(name="pvs_ps", bufs=1, space="PSUM"))

        # ---- PE warmup: dummy matmuls so the HAM clock-gate lifts
        # before the first real matmul (saves ~10us of half-clock start).
        warm = consts.tile([128, 512], f16, tag="warm")
        nc.vector.memset(warm, 0.0)
        warm_ps = mm_ps.tile([128, 512], f32, tag="mm")
        for i in range(30):
            nc.tensor.matmul(warm_ps, lhsT=warm[:, 0:128], rhs=warm,
                             start=(i == 0), stop=(i == 29))

        # ---- input DMAs (most urgent first) ----------------------------
        sl_sb = consts.tile([128, HL], f32, tag="sl")
        sl_bcast = bass.AP(tensor=sl.tensor, offset=sl.offset,
                           ap=[[0, 128], [1, HL]])
        nc.gpsimd.dma_start(out=sl_sb, in_=sl_bcast)

        # NOTE: each dma_start consumes a completion semaphore from a small
        # pool; too many outstanding DMAs serialize the ISSUES on sem reuse
        # (measured: a 13-issue input stream stalled 8us mid-kernel).  Keep
        # the input stream at <= ~9 dma_starts.
        wq_sb = consts.tile([128, 8, DL], f16, tag="wq")       # [d-chunk][kt][d']
        wq_r = wq.rearrange("(t p) m -> p t m", p=128)
        # pair-0 output-column block first: the first kt chain only needs
        # d' 0:128, so it can start ~3us earlier.
        nc.sync.dma_start(out=wq_sb[:, :, 0:128], in_=wq_r[:, :, 0:128])

        xk_t, xq_t, xv_t = {}, {}, {}

        def load_x(dst, src, half, tag):
            t = xsp.tile([128, 8, 512], f16, tag=tag)
            nc.sync.dma_start(
                out=t,
                in_=src[:, half * 512:(half + 1) * 512]
                    .rearrange("(t p) m -> p t m", p=128))
            dst[half] = t

        def load_xv(half):
            t = xvp.tile([128, 8, 512], f16, tag="xv")
            nc.sync.dma_start(
                out=t,
                in_=xv[:, half * 512:(half + 1) * 512]
                    .rearrange("(t p) m -> p t m", p=128))
            xv_t[half] = t

        load_x(xk_t, xk, 0, "xk0")
        load_x(xq_t, xq, 0, "xq0")
        load_x(xk_t, xk, 1, "xk1")
        nc.sync.dma_start(out=wq_sb[:, :, 128:512], in_=wq_r[:, :, 128:512])
        load_xv(0)

        # ---- constants -------------------------------------------------
        # V with a ones column per head ([128 s][8 st][8 h][65]); PV and
        # row-sums fuse into one M=65 matmul per head.
        v_sb = big.tile([128, 8, HL, 65], bf16, tag="v")
        ones8 = consts.tile([128, HL], bf16, tag="ones8")
        nc.vector.memset(ones8, 1.0)
        for st in range(8):
            nc.vector.tensor_copy(v_sb[:, st, :, 64], ones8)

        # qt_z: per-head Q with partitions 64-127 zeroed, so the scores
        # matmuls run at K=128 -- the whole kernel then stays in the
        # (128,128) PE tiling mode (a K=64/K=128 mode switch costs ~390ns
        # of drain per matmul, measured).
        qt_z = big.tile([128, HL, S], f16, tag="qt")
        nc.vector.memset(qt_z, 0.0)
        kt_sb = big.tile([128, 4, S], f16, tag="kt")
        ctx_sb = big.tile([128, 4, S], f16, tag="ctx")
        # out collect tile: one DMA per (ic, mt-quad) instead of 16
        # per-chain DMAs (each dma_start costs ~1.1us of sync-queue time).
        out_sb = big.tile([128, 2, 8, 512], f16, tag="osb")

        # Toeplitz exp-bias tables tp[jl, pair, hh, m] = exp(-s*max(m-jl,0))
        # generated ON-DEVICE (saves 2MB of input DMA on the critical input
        # stream): iota ramp (m - jl) -> relu -> per-head exp with the
        # per-partition slope AP as the activation scale.
        tp_sb = consts.tile([128, 4, 2, 1024], bf16, tag="tp")
        ramp_i = consts.tile([128, 1024], i32, tag="rampi")
        nc.gpsimd.iota(ramp_i, pattern=[[1, 1024]], base=0,
                       channel_multiplier=-1)
        ramp_f = consts.tile([128, 1024], f32, tag="rampf")
        nc.vector.tensor_scalar_max(ramp_f, ramp_i, 0.0)

        def gen_tp(pair):
            # 2 exps per pair, emitted shortly before the pair's first
            # group so they fill ACT idle slots instead of forming one
            # 10us block that delays the attention exp stream.
            for hh in range(2):
                h = 2 * pair + hh
                nc.scalar.activation(tp_sb[:, pair, hh, :], ramp_f, EXP,
                                     scale=sl_sb[:, h:h + 1])

        gen_tp(0)

        # ---- QT/KT projection chains (one (mt, half) chain each) ------
        def kt_chain(mt, half):
            ps = mm_ps.tile([128, 512], f32, tag="mm")
            for kt in range(8):
                nc.tensor.matmul(
                    ps,
                    lhsT=wq_sb[:, kt, mt * 128:(mt + 1) * 128],
                    rhs=xk_t[half][:, kt, :],
                    start=(kt == 0), stop=(kt == 7))
            nc.vector.tensor_copy(
                kt_sb[:, mt, half * 512:(half + 1) * 512], ps)

        def qt_chain(mt, half, on_dve=False):
            ps = mm_ps.tile([128, 512], f32, tag="mm")
            for kt in range(8):
                nc.tensor.matmul(
                    ps,
                    lhsT=wq_sb[:, kt, mt * 128:(mt + 1) * 128],
                    rhs=xq_t[half][:, kt, :],
                    start=(kt == 0), stop=(kt == 7))
            # per head, aligned to the pair rows (head 2mt -> rows 0:64,
            # head 2mt+1 -> rows 64:128; complementary rows stay zero)
            cp = nc.vector.tensor_copy if on_dve else nc.scalar.copy
            sl = slice(half * 512, (half + 1) * 512)
            cp(qt_z[0:64, 2 * mt, sl], ps[0:64, :])
            cp(qt_z[64:128, 2 * mt + 1, sl], ps[64:128, :])

        def v_proj_tile(st):
            half, q4 = st // 4, st % 4
            ps = mm_ps.tile([128, 512], f32, tag="mm")
            for kt in range(8):
                nc.tensor.matmul(
                    ps,
                    lhsT=xv_t[half][:, kt, q4 * 128:(q4 + 1) * 128],
                    rhs=wq_sb[:, kt, :],
                    start=(kt == 0), stop=(kt == 7))
            # ACT evac: group (0,0) hosts all v chains and its ACT is
            # near-idle; DVE evac there stalled the mm_ps ring behind
            # queued eb-multiplies.
            nc.scalar.copy(
                v_sb[:, st, :, 0:64],
                ps.rearrange("p (h c) -> p h c", c=64))

        # ---- attention: flat 64-step software pipeline -----------------
        # All (group, jt) steps run in one stream with the scores matmuls
        # leading the exp/PV work by 2 steps ACROSS group boundaries --
        # the per-group loop restart used to cost a ~1-2us PE bubble at
        # each of the 8 boundaries.
        def normalize(pair, ic, pvs):
            # The sums row is copied straight out of PSUM first so the
            # recip -> broadcast chain starts immediately; the bulk ctx
            # evacuation (which frees pvs for the next group) runs in
            # parallel on the other engine.
            i0 = ic * 512
            sums_sb = small.tile([1, 1024], f32, tag="sums")
            nc.vector.tensor_copy(sums_sb, pvs[64:65, :])
            pvs_sb = small.tile([64, 1024], f32, tag="pvs_sb")
            if ic == 1:   # ic1 groups are DVE-heavy; evacuate via ACT there
                nc.scalar.copy(pvs_sb, pvs[0:64, :])
            else:
                nc.vector.tensor_copy(pvs_sb, pvs[0:64, :])
            recip = small.tile([1, 1024], f32, tag="recip")
            nc.vector.reciprocal_approx_fast(recip, sums_sb)
            rb = small.tile([64, 1024], f32, tag="rb")
            nc.gpsimd.partition_broadcast(rb, recip, channels=64)
            for half, off in ((0, 0), (1, 64)):
                nc.vector.tensor_tensor(
                    out=ctx_sb[off:off + 64, pair, i0:i0 + 512],
                    in0=pvs_sb[:, half * 512:(half + 1) * 512],
                    in1=rb[:, half * 512:(half + 1) * 512], op=MULT)

        # Manual 2-slot score ring in ONE 4-bank PSUM tile: slot s%2
        # holds step s's two head-tiles.  One exp ACTIVATE then covers
        # TWO steps (4 banks, 2048 cols), halving the per-instruction
        # ACT overhead (352 cycles each) vs per-step exps.
        sc4 = sc_ps.tile([128, 4, 512], f32, tag="sc", bufs=1)

        def attn_pipeline(sched, extras):
            """sched: list of (pair, ic) groups; extras: per-group-index
            optional per-jt PE fill callback."""
            nsteps = 8 * len(sched)
            pvs_map = {}
            p4_map = {}

            def emit_scores(idx):
                pair, ic = sched[idx // 8]
                jt = idx % 8
                slot = 2 * (idx % 2)
                for half, h in ((0, 2 * pair), (1, 2 * pair + 1)):
                    nc.tensor.matmul(
                        sc4[:, slot + half, :],
                        lhsT=kt_sb[:, pair, jt * 128:(jt + 1) * 128],
                        rhs=qt_z[:, h, ic * 512:ic * 512 + 512],
                        start=True, stop=True)

            emit_scores(0)
            emit_scores(1)
            for idx in range(nsteps):
                g, jt = idx // 8, idx % 8
                pair, ic = sched[g]
                i0 = ic * 512
                if jt == 0:
                    pvs_map[g] = pvs_ps.tile([128, 1024], f32, tag="pvs",
                                             name="pvs")
                if idx % 2 == 0:
                    # exp(steps idx, idx+1) BEFORE emitting scores(idx+2),
                    # which recycles ring slot 0 (WAR tracked by region).
                    p4 = pexp.tile([128, 4, 512], bf16, tag="p", name="p4")
                    nc.scalar.activation(p4, sc4, EXP)
                    p4_map[idx // 2] = p4
                if idx + 2 < nsteps:
                    emit_scores(idx + 2)
                extra = extras.get(g)
                if extra is not None:
                    extra(jt)
                p4 = p4_map[idx // 2]
                slot = 2 * (idx % 2)
                # ALiBi: multiply by exp(bias), nonzero only for i > j:
                # columns >= c0 = max(0, j0-i0); Toeplitz offset o = i0-j0.
                o = i0 - jt * 128
                c0 = max(0, -o)
                if c0 < 512:
                    nc.vector.tensor_tensor(
                        out=p4[:, slot:slot + 2, c0:512],
                        in0=p4[:, slot:slot + 2, c0:512],
                        in1=tp_sb[:, pair, :, o + c0:o + 512],
                        op=MULT)
                # fused PV + row-sums (M=65: 64 ctx rows + sums row)
                for half, h in ((0, 2 * pair), (1, 2 * pair + 1)):
                    nc.tensor.matmul(
                        pvs_map[g][0:65, half * 512:(half + 1) * 512],
                        lhsT=v_sb[:, jt, h, :],
                        rhs=p4[:, slot + half, :],
                        start=(jt == 0), stop=(jt == 7))
                if jt == 7:
                    normalize(pair, ic, pvs_map.pop(g))

        # ---- schedule --------------------------------------------------
        # Phase 1: ic=0 attention groups; later pairs' QT/KT chains
        # interleave into earlier groups.  Group order puts (3,0) before
        # (2,1) so all ic=0 output-projection chains can interleave into
        # the last two groups; only the 8 ic=1 chains trail.
        wo_sb = consts.tile([128, 4, D], f16, tag="wo")        # [c-chunk][ct][o]

        def outproj_chain(mt, ic, ps=None):
            if ps is None:
                ps = mm_ps.tile([128, 512], f32, tag="mm")
            for ct in range(4):
                nc.tensor.matmul(
                    ps,
                    lhsT=wo_sb[:, ct, mt * 128:(mt + 1) * 128],
                    rhs=ctx_sb[:, ct, ic * 512:(ic + 1) * 512],
                    start=(ct == 0), stop=(ct == 3))
            nc.vector.tensor_copy(out_sb[:, ic, mt, :], ps)

        def out_dma(mq, ic):
            nc.sync.dma_start(
                out=out[mq * 512:(mq + 1) * 512, ic * 512:(ic + 1) * 512]
                    .rearrange("(t p) m -> p t m", p=128),
                in_=out_sb[:, ic, mq * 4:(mq + 1) * 4, :])

        from functools import partial

        def fill(chains):
            def extra(jt):
                if jt % 2 == 1 and chains:
                    chains.pop(0)()
            return extra

        kt_chain(0, 0)
        qt_chain(0, 0)
        kt_chain(0, 1)
        load_x(xq_t, xq, 1, "xq1")
        load_xv(1)
        nc.sync.dma_start(out=wo_sb, in_=wo.rearrange("(t p) m -> p t m", p=128))

        c1 = [partial(kt_chain, 1, 0), partial(kt_chain, 1, 1),
              partial(qt_chain, 1, 0, True), partial(qt_chain, 1, 1, True)]
        c2 = [partial(kt_chain, 2, 0), partial(kt_chain, 2, 1),
              partial(qt_chain, 2, 0, True), partial(qt_chain, 2, 1, True)]
        c3 = [partial(kt_chain, 3, 0), partial(kt_chain, 3, 1),
              partial(qt_chain, 3, 0, True), partial(qt_chain, 3, 1, True)]
        o0 = [partial(outproj_chain, mt, 0) for mt in range(4)]
        o1 = [partial(outproj_chain, mt, 0) for mt in range(4, 8)]

        def extra0(jt):
            v_proj_tile(jt)
            if jt == 1:
                gen_tp(1)
            if jt == 5:
                qt_chain(0, 1)

        f1, f2, f3 = fill(c1), fill(c2), fill(c3)

        def extra1(jt):
            f1(jt)
            if jt == 1:
                gen_tp(2)

        def extra2(jt):
            f2(jt)
            if jt == 1:
                gen_tp(3)

        fo1 = fill(o1)

        def extra7(jt):
            if jt == 0:
                out_dma(0, 0)
            fo1(jt)

        attn_pipeline(
            [(0, 0), (0, 1), (1, 0), (1, 1), (2, 0), (3, 0), (2, 1), (3, 1)],
            {0: extra0, 1: extra1, 2: extra2, 4: f3,
             6: fill(o0), 7: extra7})
        out_dma(1, 0)
        # Tail: all 8 ic=1 chains run ct-major over 8 concurrent PSUM
        # slots (mm + freed sc/pvs banks).  The PE is in-order, so this is
        # what lets the 24 ct<3 matmuls overlap the final group's
        # normalization latency instead of stalling at the first ct=3.
        pvt = pvs_ps.tile([128, 1024], f32, tag="pvs")
        mm_a = mm_ps.tile([128, 512], f32, tag="mm")
        mm_b = mm_ps.tile([128, 512], f32, tag="mm")
        slots = [mm_a, mm_b, sc4[:, 0, :], sc4[:, 1, :],
                 sc4[:, 2, :], sc4[:, 3, :], pvt[:, 0:512], pvt[:, 512:1024]]
        for ct in range(4):
            for mt in range(8):
                nc.tensor.matmul(
                    slots[mt],
                    lhsT=wo_sb[:, ct, mt * 128:(mt + 1) * 128],
                    rhs=ctx_sb[:, ct, 512:1024],
                    start=(ct == 0), stop=(ct == 3))
        for mt in range(8):
            if mt % 2 == 0:
                nc.scalar.copy(out_sb[:, 1, mt, :], slots[mt])
            else:
                nc.vector.tensor_copy(out_sb[:, 1, mt, :], slots[mt])
        out_dma(0, 1)
        out_dma(1, 1)

    nc.compile()
    return nc


def _get_nc():
    if "nc" not in _CACHE:
        _CACHE["nc"] = _build_nc()
    return _CACHE["nc"]


def _make_in_maps(q, k, v, Wq, Wout):
    q = np.asarray(q, dtype=np.float32)
    k = np.asarray(k, dtype=np.float32)
    v = np.asarray(v, dtype=np.float32)
    Wq = np.asarray(Wq, dtype=np.float32)
    Wout = np.asarray(Wout, dtype=np.float32)

    slopes = _alibi_slopes(H)

    in_maps = []
    for c in range(NCORES):
        b, hg = c // 2, c % 2
        in_maps.append({
            "xq": np.ascontiguousarray(q[b].T.astype(np.float16)),
            "xk": np.ascontiguousarray(k[b].T.astype(np.float16)),
            "xv": np.ascontiguousarray(v[b].T.astype(np.float16)),
            "wq": np.ascontiguousarray(
                Wq[hg * DL:(hg + 1) * DL, :].T.astype(np.float16)),
            "wo": np.ascontiguousarray(
                Wout[:, hg * DL:(hg + 1) * DL].T.astype(np.float16)),
            "sl": np.ascontiguousarray(
                -slopes[hg * HL:(hg + 1) * HL][None, :]),
        })
    return in_maps


def kernel(q, k, v, mask, Wq, Wout):
    from concourse.bass_utils import run_bass_kernel_spmd

    nc = _get_nc()
    in_maps = _make_in_maps(q, k, v, Wq, Wout)
    res = run_bass_kernel_spmd(nc, in_maps, core_ids=list(range(NCORES)))

    out = np.empty((B, S, D), dtype=np.float32)
    for b in range(B):
        out[b] = (res.results[2 * b]["out"].astype(np.float32).T
                  + res.results[2 * b + 1]["out"].astype(np.float32).T)
    return out


# revision 41
# speedup vs baseline: 1.0523x; 1.0523x over previous
"""ALiBi attention (B=4, S=1024, D=1024, H=16) on 8 TRN2 NeuronCores.

Sharding: 8 cores = 4 batches x 2 head-groups (8 heads / 512 hidden each).
Each core computes, for its (batch, head-group):
    QT = wq.T @ xqT          [512, S]   (head-dim-major, "transposed" layout)
    KT = wq.T @ xkT          [512, S]
    V  = xvT.T @ wq          [S, 512]
    per head h:  ST[j,i] = KT_h.T @ QT_h          (scores transposed)
                 P = exp(ST) * T_h[., i-j]         (post-exp Toeplitz ALiBi)
                 ctxT_h = V_h.T @ P ;  sums = 1^T @ P  (PSUM-accumulated)
                 ctxT_h *= 1/sums  (broadcast along partitions)
    outT = wo.T @ ctxT       [1024, S]  (partial output, transposed, fp16)
Host transposes each core's outT and sums the two head-group partials.

ALiBi is applied AFTER exp as a multiply by a precomputed per-head
Toeplitz table T[jl, m] = exp(-slope * max(m - jl, 0)) (bf16, DVE 2x
mode, both heads of a pair in one instruction) instead of the fp32
scalar_tensor_tensor bias-add before exp -- this halves the DVE cost
and takes the bias off the scores->exp critical path.  The two heads'
score tiles land in adjacent PSUM banks so one ACTIVATE exps 1024
columns, amortizing the ACT per-instruction overhead (352 cycles).

Matmul operands are fp16 (bf16 for P/V, which need fp32-like range), so
every matmul streams at 1 cycle/row.  Mask input is all-ones per the
problem spec (where(mask==0) is the identity), so it is not shipped.
"""

import math
from contextlib import ExitStack

import numpy as np

B, S, D = 4, 1024, 1024
H, HD = 16, 64
HL = 8          # heads per core
DL = 512        # local hidden (= HL * HD)
NCORES = 8

_CACHE = {}


def _alibi_slopes(n_head):
    main = 2 ** int(math.log2(n_head))
    m_main = 2.0 ** (-8.0 / main)
    m = m_main ** np.arange(1, 1 + main, dtype=np.float32)
    if main < n_head:
        intra = 2.0 ** (-4.0 / main)
        extra = intra ** np.arange(1, 1 + 2 * (n_head - main), 2, dtype=np.float32)
        m = np.concatenate([m, extra])
    return m.astype(np.float32)


def _build_nc():
    import concourse.bass as bass
    import concourse.mybir as mybir
    import concourse.tile as tile
    from concourse import bacc

    f32 = mybir.dt.float32
    f16 = mybir.dt.float16
    bf16 = mybir.dt.bfloat16
    i32 = mybir.dt.int32
    EXP = mybir.ActivationFunctionType.Exp
    MULT = mybir.AluOpType.mult
    MAX = mybir.AluOpType.max

    nc = bacc.Bacc("TRN2", target_bir_lowering=False, debug=False,
                   num_devices=NCORES)

    xq = nc.dram_tensor("xq", [D, S], f16, kind="ExternalInput").ap()
    xk = nc.dram_tensor("xk", [D, S], f16, kind="ExternalInput").ap()
    xv = nc.dram_tensor("xv", [D, S], f16, kind="ExternalInput").ap()
    wq = nc.dram_tensor("wq", [D, DL], f16, kind="ExternalInput").ap()
    wo = nc.dram_tensor("wo", [DL, D], f16, kind="ExternalInput").ap()
    # negated per-head ALiBi slopes (this core's 8 heads)
    sl = nc.dram_tensor("sl", [1, HL], f32, kind="ExternalInput").ap()
    out = nc.dram_tensor("out", [D, S], f16, kind="ExternalOutput").ap()

    with ExitStack() as ctx:
        tc = ctx.enter_context(tile.TileContext(nc))

        consts = ctx.enter_context(tc.tile_pool(name="consts", bufs=1))
        xvp = ctx.enter_context(tc.tile_pool(name="xvp", bufs=1))
        xsp = ctx.enter_context(tc.tile_pool(name="xsp", bufs=1))
        big = ctx.enter_context(tc.tile_pool(name="big", bufs=1))
        pexp = ctx.enter_context(tc.tile_pool(name="pexp", bufs=3))
        small = ctx.enter_context(tc.tile_pool(name="small", bufs=2))
        mm_ps = ctx.enter_context(tc.tile_pool(name="mm_ps", bufs=2, space="PSUM"))
        sc_ps = ctx.enter_context(tc.tile_pool(name="sc_ps", bufs=2, space="PSUM"))
        pvs_ps = ctx.enter_context(tc.tile_pool(name="pvs_ps", bufs=1, space="PSUM"))

        # ---- PE warmup: dummy matmuls so the HAM clock-gate lifts
        # before the first real matmul (saves ~10us of half-clock start).
        warm = consts.tile([128, 512], f16, tag="warm")
        nc.vector.memset(warm, 0.0)
        warm_ps = mm_ps.tile([128, 512], f32, tag="mm")
        for i in range(30):
            nc.tensor.matmul(warm_ps, lhsT=warm[:, 0:128], rhs=warm,
                             start=(i == 0), stop=(i == 29))

        # ---- input DMAs (most urgent first) ----------------------------
        sl_sb = consts.tile([128, HL], f32, tag="sl")
        sl_bcast = bass.AP(tensor=sl.tensor, offset=sl.offset,
                           ap=[[0, 128], [1, HL]])
        nc.gpsimd.dma_start(out=sl_sb, in_=sl_bcast)

        # NOTE: each dma_start consumes a completion semaphore from a small
        # pool; too many outstanding DMAs serialize the ISSUES on sem reuse
        # (measured: a 13-issue input stream stalled 8us mid-kernel).  Keep
        # the input stream at <= ~9 dma_starts.
        wq_sb = consts.tile([128, 8, DL], f16, tag="wq")       # [d-chunk][kt][d']
        wq_r = wq.rearrange("(t p) m -> p t m", p=128)
        # pair-0 output-column block first: the first kt chain only needs
        # d' 0:128, so it can start ~3us earlier.
        nc.sync.dma_start(out=wq_sb[:, :, 0:128], in_=wq_r[:, :, 0:128])

        xk_t, xq_t, xv_t = {}, {}, {}

        def load_x(dst, src, half, tag):
            t = xsp.tile([128, 8, 512], f16, tag=tag)
            nc.sync.dma_start(
                out=t,
                in_=src[:, half * 512:(half + 1) * 512]
                    .rearrange("(t p) m -> p t m", p=128))
            dst[half] = t

        def load_xv(half):
            t = xvp.tile([128, 8, 512], f16, tag="xv")
            nc.sync.dma_start(
                out=t,
                in_=xv[:, half * 512:(half + 1) * 512]
                    .rearrange("(t p) m -> p t m", p=128))
            xv_t[half] = t

        load_x(xk_t, xk, 0, "xk0")
        load_x(xq_t, xq, 0, "xq0")
        load_x(xk_t, xk, 1, "xk1")
        nc.sync.dma_start(out=wq_sb[:, :, 128:512], in_=wq_r[:, :, 128:512])
        load_xv(0)

        # ---- constants -------------------------------------------------
        # V with a ones column per head ([128 s][8 st][8 h][65]); PV and
        # row-sums fuse into one M=65 matmul per head.
        v_sb = big.tile([128, 8, HL, 65], bf16, tag="v")
        ones8 = consts.tile([128, HL], bf16, tag="ones8")
        nc.vector.memset(ones8, 1.0)
        for st in range(8):
            nc.vector.tensor_copy(v_sb[:, st, :, 64], ones8)

        # qt_z: per-head Q with partitions 64-127 zeroed, so the scores
        # matmuls run at K=128 -- the whole kernel then stays in the
        # (128,128) PE tiling mode (a K=64/K=128 mode switch costs ~390ns
        # of drain per matmul, measured).
        qt_z = big.tile([128, HL, S], f16, tag="qt")
        nc.vector.memset(qt_z, 0.0)
        kt_sb = big.tile([128, 4, S], f16, tag="kt")
        ctx_sb = big.tile([128, 4, S], f16, tag="ctx")
        # out collect tile: one DMA per (ic, mt-quad) instead of 16
        # per-chain DMAs (each dma_start costs ~1.1us of sync-queue time).
        out_sb = big.tile([128, 2, 8, 512], f16, tag="osb")

        # Toeplitz exp-bias tables tp[jl, pair, hh, m] = exp(-s*max(m-jl,0))
        # generated ON-DEVICE (saves 2MB of input DMA on the critical input
        # stream): iota ramp (m - jl) -> relu -> per-head exp with the
        # per-partition slope AP as the activation scale.
        tp_sb = consts.tile([128, 4, 2, 1024], bf16, tag="tp")
        ramp_i = consts.tile([128, 1024], i32, tag="rampi")
        nc.gpsimd.iota(ramp_i, pattern=[[1, 1024]], base=0,
                       channel_multiplier=-1)
        ramp_f = consts.tile([128, 1024], f32, tag="rampf")
        nc.vector.tensor_scalar_max(ramp_f, ramp_i, 0.0)

        def gen_tp(pair):
            # 2 exps per pair, emitted shortly before the pair's first
            # group so they fill ACT idle slots instead of forming one
            # 10us block that delays the attention exp stream.
            for hh in range(2):
                h = 2 * pair + hh
                nc.scalar.activation(tp_sb[:, pair, hh, :], ramp_f, EXP,
                                     scale=sl_sb[:, h:h + 1])

        gen_tp(0)

        # ---- QT/KT projection chains (one (mt, half) chain each) ------
        def kt_chain(mt, half):
            ps = mm_ps.tile([128, 512], f32, tag="mm")
            for kt in range(8):
                nc.tensor.matmul(
                    ps,
                    lhsT=wq_sb[:, kt, mt * 128:(mt + 1) * 128],
                    rhs=xk_t[half][:, kt, :],
                    start=(kt == 0), stop=(kt == 7))
            nc.vector.tensor_copy(
                kt_sb[:, mt, half * 512:(half + 1) * 512], ps)

        def qt_chain(mt, half, on_dve=False):
            ps = mm_ps.tile([128, 512], f32, tag="mm")
            for kt in range(8):
                nc.tensor.matmul(
                    ps,
                    lhsT=wq_sb[:, kt, mt * 128:(mt + 1) * 128],
                    rhs=xq_t[half][:, kt, :],
                    start=(kt == 0), stop=(kt == 7))
            # per head, aligned to the pair rows (head 2mt -> rows 0:64,
            # head 2mt+1 -> rows 64:128; complementary rows stay zero)
            cp = nc.vector.tensor_copy if on_dve else nc.scalar.copy
            sl = slice(half * 512, (half + 1) * 512)
            cp(qt_z[0:64, 2 * mt, sl], ps[0:64, :])
            cp(qt_z[64:128, 2 * mt + 1, sl], ps[64:128, :])

        def v_proj_tile(st):
            half, q4 = st // 4, st % 4
            ps = mm_ps.tile([128, 512], f32, tag="mm")
            for kt in range(8):
                nc.tensor.matmul(
                    ps,
                    lhsT=xv_t[half][:, kt, q4 * 128:(q4 + 1) * 128],
                    rhs=wq_sb[:, kt, :],
                    start=(kt == 0), stop=(kt == 7))
            # ACT evac: group (0,0) hosts all v chains and its ACT is
            # near-idle; DVE evac there stalled the mm_ps ring behind
            # queued eb-multiplies.
            nc.scalar.copy(
                v_sb[:, st, :, 0:64],
                ps.rearrange("p (h c) -> p h c", c=64))

        # ---- attention: flat 64-step software pipeline -----------------
        # All (group, jt) steps run in one stream with the scores matmuls
        # leading the exp/PV work by 2 steps ACROSS group boundaries --
        # the per-group loop restart used to cost a ~1-2us PE bubble at
        # each of the 8 boundaries.
        def normalize(pair, ic, pvs):
            # The sums row is copied straight out of PSUM first so the
            # recip -> broadcast chain starts immediately; the bulk ctx
            # evacuation (which frees pvs for the next group) runs in
            # parallel on the other engine.
            i0 = ic * 512
            sums_sb = small.tile([1, 1024], f32, tag="sums")
            nc.vector.tensor_copy(sums_sb, pvs[64:65, :])
            pvs_sb = small.tile([64, 1024], f32, tag="pvs_sb")
            if ic == 1:   # ic1 groups are DVE-heavy; evacuate via ACT there
                nc.scalar.copy(pvs_sb, pvs[0:64, :])
            else:
                nc.vector.tensor_copy(pvs_sb, pvs[0:64, :])
            recip = small.tile([1, 1024], f32, tag="recip")
            nc.vector.reciprocal_approx_fast(recip, sums_sb)
            rb = small.tile([64, 1024], f32, tag="rb")
            nc.gpsimd.partition_broadcast(rb, recip, channels=64)
            for half, off in ((0, 0), (1, 64)):
                nc.vector.tensor_tensor(
                    out=ctx_sb[off:off + 64, pair, i0:i0 + 512],
                    in0=pvs_sb[:, half * 512:(half + 1) * 512],
                    in1=rb[:, half * 512:(half + 1) * 512], op=MULT)

        def attn_pipeline(sched, extras):
            """sched: list of (pair, ic) groups; extras: per-group-index
            optional per-jt PE fill callback."""
            nsteps = 8 * len(sched)
            sc_tiles = {}
            pvs_map = {}

            def emit_scores(idx):
                pair, ic = sched[idx // 8]
                jt = idx % 8
                sc = sc_ps.tile([128, 2, 512], f32, tag="sc")
                for half, h in ((0, 2 * pair), (1, 2 * pair + 1)):
                    nc.tensor.matmul(
                        sc[:, half, :],
                        lhsT=kt_sb[:, pair, jt * 128:(jt + 1) * 128],
                        rhs=qt_z[:, h, ic * 512:ic * 512 + 512],
                        start=True, stop=True)
                sc_tiles[idx] = sc

            emit_scores(0)
            emit_scores(1)
            for idx in range(nsteps):
                g, jt = idx // 8, idx % 8
                pair, ic = sched[g]
                i0 = ic * 512
                if jt == 0:
                    pvs_map[g] = pvs_ps.tile([128, 1024], f32, tag="pvs",
                                             name="pvs")
                if idx + 2 < nsteps:
                    emit_scores(idx + 2)
                extra = extras.get(g)
                if extra is not None:
                    extra(jt)
                # one exp over both heads' score tiles (adjacent banks)
                p2 = pexp.tile([128, 2, 512], bf16, tag="p")
                nc.scalar.activation(p2, sc_tiles.pop(idx), EXP)
                # ALiBi: multiply by exp(bias), nonzero only for i > j:
                # columns >= c0 = max(0, j0-i0); Toeplitz offset o = i0-j0.
                o = i0 - jt * 128
                c0 = max(0, -o)
                if c0 < 512:
                    nc.vector.tensor_tensor(
                        out=p2[:, :, c0:512],
                        in0=p2[:, :, c0:512],
                        in1=tp_sb[:, pair, :, o + c0:o + 512],
                        op=MULT)
                # fused PV + row-sums (M=65: 64 ctx rows + sums row)
                for half, h in ((0, 2 * pair), (1, 2 * pair + 1)):
                    nc.tensor.matmul(
                        pvs_map[g][0:65, half * 512:(half + 1) * 512],
                        lhsT=v_sb[:, jt, h, :],
                        rhs=p2[:, half, :],
                        start=(jt == 0), stop=(jt == 7))
                if jt == 7:
                    normalize(pair, ic, pvs_map.pop(g))

        # ---- schedule --------------------------------------------------
        # Phase 1: ic=0 attention groups; later pairs' QT/KT chains
        # interleave into earlier groups.  Group order puts (3,0) before
        # (2,1) so all ic=0 output-projection chains can interleave into
        # the last two groups; only the 8 ic=1 chains trail.
        wo_sb = consts.tile([128, 4, D], f16, tag="wo")        # [c-chunk][ct][o]

        def outproj_chain(mt, ic, ps=None):
            if ps is None:
                ps = mm_ps.tile([128, 512], f32, tag="mm")
            for ct in range(4):
                nc.tensor.matmul(
                    ps,
                    lhsT=wo_sb[:, ct, mt * 128:(mt + 1) * 128],
                    rhs=ctx_sb[:, ct, ic * 512:(ic + 1) * 512],
                    start=(ct == 0), stop=(ct == 3))
            nc.vector.tensor_copy(out_sb[:, ic, mt, :], ps)

        def out_dma(mq, ic):
            nc.sync.dma_start(
                out=out[mq * 512:(mq + 1) * 512, ic * 512:(ic + 1) * 512]
                    .rearrange("(t p) m -> p t m", p=128),
                in_=out_sb[:, ic, mq * 4:(mq + 1) * 4, :])

        from functools import partial

        def fill(chains):
            def extra(jt):
                if jt % 2 == 1 and chains:
                    chains.pop(0)()
            return extra

        kt_chain(0, 0)
        qt_chain(0, 0)
        kt_chain(0, 1)
        load_x(xq_t, xq, 1, "xq1")
        load_xv(1)
        nc.sync.dma_start(out=wo_sb, in_=wo.rearrange("(t p) m -> p t m", p=128))

        c1 = [partial(kt_chain, 1, 0), partial(kt_chain, 1, 1),
              partial(qt_chain, 1, 0, True), partial(qt_chain, 1, 1, True)]
        c2 = [partial(kt_chain, 2, 0), partial(kt_chain, 2, 1),
              partial(qt_chain, 2, 0, True), partial(qt_chain, 2, 1, True)]
        c3 = [partial(kt_chain, 3, 0), partial(kt_chain, 3, 1),
              partial(qt_chain, 3, 0, True), partial(qt_chain, 3, 1, True)]
        o0 = [partial(outproj_chain, mt, 0) for mt in range(4)]
        o1 = [partial(outproj_chain, mt, 0) for mt in range(4, 8)]

        def extra0(jt):
            v_proj_tile(jt)
            if jt == 1:
                gen_tp(1)
            if jt == 5:
                qt_chain(0, 1)

        f1, f2, f3 = fill(c1), fill(c2), fill(c3)

        def extra1(jt):
            f1(jt)
            if jt == 1:
                gen_tp(2)

        def extra2(jt):
            f2(jt)
            if jt == 1:
                gen_tp(3)

        fo1 = fill(o1)

        def extra7(jt):
            if jt == 0:
                out_dma(0, 0)
            fo1(jt)

        attn_pipeline(
            [(0, 0), (0, 1), (1, 0), (1, 1), (2, 0), (3, 0), (2, 1), (3, 1)],
            {0: extra0, 1: extra1, 2: extra2, 4: f3,
             6: fill(o0), 7: extra7})
        out_dma(1, 0)
        # Tail: all 8 ic=1 chains run ct-major over 8 concurrent PSUM
        # slots (mm + freed sc/pvs banks).  The PE is in-order, so this is
        # what lets the 24 ct<3 matmuls overlap the final group's
        # normalization latency instead of stalling at the first ct=3.
        sc_a = sc_ps.tile([128, 2, 512], f32, tag="sc")
        sc_b = sc_ps.tile([128, 2, 512], f32, tag="sc")
        pvt = pvs_ps.tile([128, 1024], f32, tag="pvs")
        mm_a = mm_ps.tile([128, 512], f32, tag="mm")
        mm_b = mm_ps.tile([128, 512], f32, tag="mm")
        slots = [mm_a, mm_b, sc_a[:, 0, :], sc_a[:, 1, :],
                 sc_b[:, 0, :], sc_b[:, 1, :], pvt[:, 0:512], pvt[:, 512:1024]]
        for ct in range(4):
            for mt in range(8):
                nc.tensor.matmul(
                    slots[mt],
                    lhsT=wo_sb[:, ct, mt * 128:(mt + 1) * 128],
                    rhs=ctx_sb[:, ct, 512:1024],
                    start=(ct == 0), stop=(ct == 3))
        for mt in range(8):
            if mt % 2 == 0:
                nc.scalar.copy(out_sb[:, 1, mt, :], slots[mt])
            else:
                nc.vector.tensor_copy(out_sb[:, 1, mt, :], slots[mt])
        out_dma(0, 1)
        out_dma(1, 1)

    nc.compile()
    return nc


def _get_nc():
    if "nc" not in _CACHE:
        _CACHE["nc"] = _build_nc()
    return _CACHE["nc"]


def _make_in_maps(q, k, v, Wq, Wout):
    q = np.asarray(q, dtype=np.float32)
    k = np.asarray(k, dtype=np.float32)
    v = np.asarray(v, dtype=np.float32)
    Wq = np.asarray(Wq, dtype=np.float32)
    Wout = np.asarray(Wout, dtype=np.float32)

    slopes = _alibi_slopes(H)

    in_maps = []
    for c in range(NCORES):
        b, hg = c // 2, c % 2
        in_maps.append({
            "xq": np.ascontiguousarray(q[b].T.astype(np.float16)),
            "xk": np.ascontiguousarray(k[b].T.astype(np.float16)),
            "xv": np.ascontiguousarray(v[b].T.astype(np.float16)),
            "wq": np.ascontiguousarray(
                Wq[hg * DL:(hg + 1) * DL, :].T.astype(np.float16)),
            "wo": np.ascontiguousarray(
                Wout[:, hg * DL:(hg + 1) * DL].T.astype(np.float16)),
            "sl": np.ascontiguousarray(
                -slopes[hg * HL:(hg + 1) * HL][None, :]),
        })
    return in_maps


def kernel(q, k, v, mask, Wq, Wout):
    from concourse.bass_utils import run_bass_kernel_spmd

    nc = _get_nc()
    in_maps = _make_in_maps(q, k, v, Wq, Wout)
    res = run_bass_kernel_spmd(nc, in_maps, core_ids=list(range(NCORES)))

    out = np.empty((B, S, D), dtype=np.float32)
    for b in range(B):
        out[b] = (res.results[2 * b]["out"].astype(np.float32).T
                  + res.results[2 * b + 1]["out"].astype(np.float32).T)
    return out
